# revision 4
# baseline (speedup 1.0000x reference)
"""Trainium2 Bass kernel for the MoE-routing model (ModelInternalClassSelection).

Contract: kernel(**inputs) takes the FULL fp32 inputs (keyed as in
setup_inputs) and returns the FULL outputs (out, ws, vs_stacked), matching
the jax reference:

    x1  = relu(x @ W1 + b1)                      # [B, H]
    g   = relu(x1 @ Wg1 + bg1)                   # [B, G]
    ws  = softmax(g @ Wg2 + bg2, axis=1)         # [B, E]
    h   = relu(einsum('bh,ehd->bed', x1, We1) + be1)
    vs  = einsum('bed,edf->bef', h, We2) + be2   # [B, E, D]
    out = einsum('be,bed->bd', ws, vs)           # [B, D]
    return out, ws, transpose(vs, (0, 2, 1))     # vs_stacked: [B, D, E]

Strategy: data-parallel over the batch — 8 NeuronCores, 1024 rows each;
expert/gating weights replicated. On-device everything is computed in the
"transposed activation" layout (features on SBUF partitions, batch on the
free dim) so weights in their natural [K_in, K_out] layout are the matmul
stationary operand and no activation transposes are needed. Matmul inputs
are bf16 (fp32 PSUM accumulation); outputs are fp32.
"""

import sys

sys.path.insert(0, "/opt/trn_rl_repo")

import numpy as np
import ml_dtypes

import bass_rust
import concourse.bass as bass
import concourse.mybir as mybir
import concourse.tile as tile
from concourse.vector_clock import ScopedClock
from concourse.bass_utils import run_bass_kernel_spmd

BF16 = ml_dtypes.bfloat16

# Model dims (hardcoded per the problem spec).
B = 8192
C_IN = 2048
H = 1024
G = 512
E = 16
D = 512

N_CORES = 8
BS = B // N_CORES      # 1024 batch rows per core
P = 128                # SBUF partitions
NFREE = 512            # matmul moving-operand free-dim chunk
NB = BS // NFREE       # batch chunks per core (2)

F32 = mybir.dt.float32
BF16_T = mybir.dt.bfloat16


class SplitDrainTileContext(tile.TileContext):
    """TileContext whose exit drain splits its sem waits one-per-Drain.

    The walrus codegen in this container rejects a Drain carrying more than
    one sync wait ("Too many sync wait commands"); the stock TileContext
    attaches the whole global clock to a single kernel-tail Drain.
    """

    def _drain_and_barrier(self, tick_clock, wait_clock):
        drain_inst = self.nc.sync.drain()
        wait_clock.add_sem_waits(
            drain_inst.ins, ScopedClock({None: tick_clock.global_clock})
        )
        waits = list(drain_inst.ins.sync_info.on_wait)
        if len(waits) > 1:
            drain_inst.ins.sync_info.on_wait = waits[:1]
            for i in range(1, len(waits)):
                extra = self.nc.sync.drain()
                extra.ins.sync_info = mybir.SyncInfo(
                    on_wait=waits[i : i + 1], on_update=[]
                )
        self.nc.all_engine_barrier()
        assert self.sems is not None
        popped = self.nc._tile_sem_poison_stack.pop()
        assert popped is self._sem_poison
        self.nc.clear_and_free_semaphores(list(self.sems.allocated().values()))
        self.nc.all_engine_barrier()


def _split_sync_waits(nc: bass.Bass, max_waits: int = 1) -> None:
    """Hoist excess per-instruction sem waits onto same-engine NoOps.

    The walrus codegen in this container rejects instructions carrying more
    than one sync wait. A NoOp-with-wait immediately before the instruction
    on the same engine enforces the identical ordering.
    """
    for f in nc.m.functions:
        for bb in f.blocks:
            new_insts = []
            for inst in bb.instructions:
                si = inst.sync_info
                waits = list(si.on_wait) if si and si.on_wait else []
                if len(waits) > max_waits:
                    keep = waits[-max_waits:]
                    hoist = waits[:-max_waits]
                    for i in range(0, len(hoist), max_waits):
                        nop = bass_rust.InstNoOp(
                            name=f"{inst.name}-sw{i}", engine=inst.engine
                        )
                        nop.sync_info = mybir.SyncInfo(
                            on_wait=hoist[i : i + max_waits], on_update=[]
                        )
                        new_insts.append(nop)
                    inst.sync_info = mybir.SyncInfo(
                        on_wait=keep,
                        on_update=list(si.on_update) if si.on_update else [],
                    )
                new_insts.append(inst)
            bb.instructions = new_insts


def build_program() -> bass.Bass:
    nc = bass.Bass("TRN2", target_bir_lowering=False, debug=False, num_devices=N_CORES)

    # Per-core inputs (bf16 compute operands, fp32 biases/identity).
    xt = nc.declare_dram_parameter("xt", [C_IN, BS], BF16_T, isOutput=False)
    w1 = nc.declare_dram_parameter("w1", [C_IN, H], BF16_T, isOutput=False)
    wg1 = nc.declare_dram_parameter("wg1", [H, G], BF16_T, isOutput=False)
    wg2 = nc.declare_dram_parameter("wg2", [G, E], BF16_T, isOutput=False)
    we1 = nc.declare_dram_parameter("we1", [E, H, D], BF16_T, isOutput=False)
    we2 = nc.declare_dram_parameter("we2", [E, D, D], BF16_T, isOutput=False)
    b1t = nc.declare_dram_parameter("b1t", [P, H // P], F32, isOutput=False)
    bg1t = nc.declare_dram_parameter("bg1t", [P, G // P], F32, isOutput=False)
    bg2b = nc.declare_dram_parameter("bg2b", [P, E], F32, isOutput=False)
    be1t = nc.declare_dram_parameter("be1t", [P, E, D // P], F32, isOutput=False)
    be2t = nc.declare_dram_parameter("be2t", [P, E, D // P], F32, isOutput=False)
    idn = nc.declare_dram_parameter("idn", [P, P], F32, isOutput=False)

    # Per-core outputs.
    vst = nc.declare_dram_parameter("vst", [E, D, BS], F32, isOutput=True)
    outt = nc.declare_dram_parameter("outt", [D, BS], F32, isOutput=True)
    ws_out = nc.declare_dram_parameter("ws", [BS, E], F32, isOutput=True)

    KC = C_IN // P   # 16 k-tiles for fc1
    KH = H // P      # 8 k-tiles for H-contraction
    KG = G // P      # 4 k-tiles for G/D-contraction
    MH = H // P      # 8 m-tiles of x1T
    MG = G // P      # 4 m-tiles of gT
    MD = D // P      # 4 m-tiles of hT/vT
    TB = BS // P     # 8 batch tiles (for softmax path)

    with SplitDrainTileContext(nc) as tc:
        with (
            tc.tile_pool(name="const", bufs=1) as const_pool,
            tc.tile_pool(name="big", bufs=1) as big_pool,
            tc.tile_pool(name="wexp", bufs=3) as wexp_pool,
            tc.tile_pool(name="hexp", bufs=2) as hexp_pool,
            tc.tile_pool(name="vsb", bufs=6) as v_pool,
            tc.tile_pool(name="wsb", bufs=2) as wsb_pool,
            tc.tile_pool(name="tmp", bufs=6) as tmp_pool,
            tc.tile_pool(name="sm", bufs=10) as sm_pool,
            tc.tile_pool(name="mmps", bufs=4, space="PSUM") as mm_psum,
            tc.tile_pool(name="lgps", bufs=2, space="PSUM") as lg_psum,
            tc.tile_pool(name="trps", bufs=2, space="PSUM") as tr_psum,
            tc.tile_pool(name="dram", bufs=1, space="DRAM") as dram_pool,
        ):
            # ---- constants / biases ----
            b1_sb = const_pool.tile([P, H // P], F32)
            nc.sync.dma_start(b1_sb[:], b1t[:])
            bg1_sb = const_pool.tile([P, G // P], F32)
            nc.sync.dma_start(bg1_sb[:], bg1t[:])
            bg2_sb = const_pool.tile([P, E], F32)
            nc.sync.dma_start(bg2_sb[:], bg2b[:])
            be1_sb = const_pool.tile([P, E, D // P], F32)
            nc.sync.dma_start(be1_sb[:], be1t[:])
            be2_sb = const_pool.tile([P, E, D // P], F32)
            nc.sync.dma_start(be2_sb[:], be2t[:])
            idn_sb = const_pool.tile([P, P], F32)
            nc.sync.dma_start(idn_sb[:], idn[:])

            # ---- stage inputs ----
            xt_sb = big_pool.tile([P, KC, BS], BF16_T)
            nc.sync.dma_start(xt_sb[:], xt.rearrange("(k p) b -> p k b", p=P))
            w1_sb = big_pool.tile([P, KC, H], BF16_T)
            nc.sync.dma_start(w1_sb[:], w1.rearrange("(k p) h -> p k h", p=P))
            wg1_sb = big_pool.tile([P, KH, G], BF16_T)
            nc.sync.dma_start(wg1_sb[:], wg1.rearrange("(k p) g -> p k g", p=P))
            wg2_sb = big_pool.tile([P, KG, E], BF16_T)
            nc.sync.dma_start(wg2_sb[:], wg2.rearrange("(k p) e -> p k e", p=P))

            # ---- fc1: x1T = relu(W1^T @ xT + b1) ----
            x1t_sb = big_pool.tile([P, MH, BS], BF16_T)
            for m in range(MH):
                for n in range(NB):
                    ps = mm_psum.tile([P, NFREE], F32)
                    for k in range(KC):
                        nc.tensor.matmul(
                            ps[:],
                            w1_sb[:, k, m * P : (m + 1) * P],
                            xt_sb[:, k, n * NFREE : (n + 1) * NFREE],
                            start=(k == 0),
                            stop=(k == KC - 1),
                        )
                    nc.scalar.activation(
                        x1t_sb[:, m, n * NFREE : (n + 1) * NFREE],
                        ps[:],
                        mybir.ActivationFunctionType.Relu,
                        bias=b1_sb[:, m : m + 1],
                    )

            # ---- gating MLP layer 1: gT = relu(Wg1^T @ x1T + bg1) ----
            gt_sb = big_pool.tile([P, MG, BS], BF16_T)
            for m in range(MG):
                for n in range(NB):
                    ps = mm_psum.tile([P, NFREE], F32)
                    for k in range(KH):
                        nc.tensor.matmul(
                            ps[:],
                            wg1_sb[:, k, m * P : (m + 1) * P],
                            x1t_sb[:, k, n * NFREE : (n + 1) * NFREE],
                            start=(k == 0),
                            stop=(k == KH - 1),
                        )
                    nc.scalar.activation(
                        gt_sb[:, m, n * NFREE : (n + 1) * NFREE],
                        ps[:],
                        mybir.ActivationFunctionType.Relu,
                        bias=bg1_sb[:, m : m + 1],
                    )

            # ---- gating logits + softmax, batch-major per 128-row tile ----
            # logits_b[b, e] via lhsT = gT column block (stationary), rhs = Wg2.
            wst_sb = big_pool.tile([E, BS], F32)
            for t in range(TB):
                lp = lg_psum.tile([P, E], F32)
                for k in range(KG):
                    nc.tensor.matmul(
                        lp[:],
                        gt_sb[:, k, t * P : (t + 1) * P],
                        wg2_sb[:, k, :],
                        start=(k == 0),
                        stop=(k == KG - 1),
                    )
                logits_sb = sm_pool.tile([P, E], F32, tag="logits")
                nc.vector.tensor_add(logits_sb[:], lp[:], bg2_sb[:])
                neg_mx = sm_pool.tile([P, 1], F32, tag="mx")
                nc.vector.tensor_reduce(
                    neg_mx[:],
                    logits_sb[:],
                    op=mybir.AluOpType.max,
                    axis=mybir.AxisListType.X,
                    negate=True,
                )
                exp_sb = sm_pool.tile([P, E], F32, tag="exp")
                ssum = sm_pool.tile([P, 1], F32, tag="ssum")
                nc.scalar.activation(
                    exp_sb[:],
                    logits_sb[:],
                    mybir.ActivationFunctionType.Exp,
                    bias=neg_mx[:],
                    accum_out=ssum[:],
                )
                inv = sm_pool.tile([P, 1], F32, tag="inv")
                nc.vector.reciprocal(inv[:], ssum[:])
                ws_b = sm_pool.tile([P, E], F32, tag="wsb")
                nc.vector.tensor_scalar_mul(ws_b[:], exp_sb[:], inv[:])
                nc.sync.dma_start(ws_out[t * P : (t + 1) * P, :], ws_b[:])
                # Transpose back to expert-major for the weighted sum.
                tp = tr_psum.tile([E, P], F32)
                nc.tensor.transpose(tp[:], ws_b[:], idn_sb[:])
                nc.scalar.activation(
                    wst_sb[:, t * P : (t + 1) * P],
                    tp[:],
                    mybir.ActivationFunctionType.Copy,
                )
            wst_dram = dram_pool.tile([E, BS], F32)
            nc.sync.dma_start(wst_dram[:], wst_sb[:])

            # ---- expert branches ----
            out_acc = big_pool.tile([P, MD, BS], F32)
            for e in range(E):
                we1_sb = wexp_pool.tile([P, KH, D], BF16_T, tag="we1")
                nc.sync.dma_start(
                    we1_sb[:], we1[e].rearrange("(k p) d -> p k d", p=P)
                )
                we2_sb = wexp_pool.tile([P, KG, D], BF16_T, tag="we2")
                nc.sync.dma_start(
                    we2_sb[:], we2[e].rearrange("(k p) d -> p k d", p=P)
                )
                # Broadcast ws[:, e] across all partitions via a DRAM read.
                ws_bc = wsb_pool.tile([P, BS], F32)
                nc.sync.dma_start(
                    ws_bc[:], wst_dram[e : e + 1, :].to_broadcast((P, BS))
                )

                ht_sb = hexp_pool.tile([P, MD, BS], BF16_T)
                for m in range(MD):
                    for n in range(NB):
                        ps = mm_psum.tile([P, NFREE], F32)
                        for k in range(KH):
                            nc.tensor.matmul(
                                ps[:],
                                we1_sb[:, k, m * P : (m + 1) * P],
                                x1t_sb[:, k, n * NFREE : (n + 1) * NFREE],
                                start=(k == 0),
                                stop=(k == KH - 1),
                            )
                        nc.scalar.activation(
                            ht_sb[:, m, n * NFREE : (n + 1) * NFREE],
                            ps[:],
                            mybir.ActivationFunctionType.Relu,
                            bias=be1_sb[:, e, m : m + 1],
                        )
                for m in range(MD):
                    for n in range(NB):
                        ps = mm_psum.tile([P, NFREE], F32)
                        for k in range(KG):
                            nc.tensor.matmul(
                                ps[:],
                                we2_sb[:, k, m * P : (m + 1) * P],
                                ht_sb[:, k, n * NFREE : (n + 1) * NFREE],
                                start=(k == 0),
                                stop=(k == KG - 1),
                            )
                        v_sb = v_pool.tile([P, NFREE], F32)
                        nc.scalar.activation(
                            v_sb[:],
                            ps[:],
                            mybir.ActivationFunctionType.Identity,
                            bias=be2_sb[:, e, m : m + 1],
                        )
                        nc.sync.dma_start(
                            vst[e, m * P : (m + 1) * P, n * NFREE : (n + 1) * NFREE],
                            v_sb[:],
                        )
                        nsl = slice(n * NFREE, (n + 1) * NFREE)
                        if e == 0:
                            nc.vector.tensor_mul(
                                out_acc[:, m, nsl], v_sb[:], ws_bc[:, nsl]
                            )
                        else:
                            tmp = tmp_pool.tile([P, NFREE], F32)
                            nc.vector.tensor_mul(tmp[:], v_sb[:], ws_bc[:, nsl])
                            nc.gpsimd.tensor_add(
                                out_acc[:, m, nsl], out_acc[:, m, nsl], tmp[:]
                            )

            nc.sync.dma_start(outt.rearrange("(m p) b -> p m b", p=P), out_acc[:])

    _split_sync_waits(nc, max_waits=1)
    return nc


def make_in_maps(x, W1, b1, Wg1, bg1, Wg2, bg2, We1, be1, We2, be2):
    """Shard + lay out the full fp32 inputs into per-core in_maps."""
    xb = x.astype(BF16)

    shared = {
        "w1": W1.astype(BF16),
        "wg1": Wg1.astype(BF16),
        "wg2": Wg2.astype(BF16),
        "we1": We1.astype(BF16),
        "we2": We2.astype(BF16),
        "b1t": np.ascontiguousarray(b1.reshape(H // P, P).T.astype(np.float32)),
        "bg1t": np.ascontiguousarray(bg1.reshape(G // P, P).T.astype(np.float32)),
        "bg2b": np.ascontiguousarray(
            np.broadcast_to(bg2.astype(np.float32), (P, E))
        ),
        "be1t": np.ascontiguousarray(
            be1.astype(np.float32).reshape(E, D // P, P).transpose(2, 0, 1)
        ),
        "be2t": np.ascontiguousarray(
            be2.astype(np.float32).reshape(E, D // P, P).transpose(2, 0, 1)
        ),
        "idn": np.eye(P, dtype=np.float32),
    }
    in_maps = []
    for c in range(N_CORES):
        m = dict(shared)
        m["xt"] = np.ascontiguousarray(xb[c * BS : (c + 1) * BS, :].T)
        in_maps.append(m)
    return in_maps


def assemble_outputs(results):
    out = np.empty((B, D), dtype=np.float32)
    ws = np.empty((B, E), dtype=np.float32)
    vs_stacked = np.empty((B, D, E), dtype=np.float32)
    for c in range(N_CORES):
        r = results[c]
        out[c * BS : (c + 1) * BS] = r["outt"].T
        ws[c * BS : (c + 1) * BS] = r["ws"]
        vs_stacked[c * BS : (c + 1) * BS] = r["vst"].transpose(2, 1, 0)
    return out, ws, vs_stacked


_PROGRAM_CACHE = {}


def get_program() -> bass.Bass:
    if "nc" not in _PROGRAM_CACHE:
        _PROGRAM_CACHE["nc"] = build_program()
    return _PROGRAM_CACHE["nc"]


def kernel(x, W1, b1, Wg1, bg1, Wg2, bg2, We1, be1, We2, be2):
    x = np.asarray(x)
    in_maps = make_in_maps(
        x,
        np.asarray(W1), np.asarray(b1),
        np.asarray(Wg1), np.asarray(bg1),
        np.asarray(Wg2), np.asarray(bg2),
        np.asarray(We1), np.asarray(be1),
        np.asarray(We2), np.asarray(be2),
    )
    nc = get_program()
    res = run_bass_kernel_spmd(nc, in_maps, list(range(N_CORES)), trace=False)
    return assemble_outputs(res.results)


if __name__ == "__main__":
    # Quick self-check with random inputs (not the reference values).
    rng = np.random.default_rng(0)
    ins = {
        "x": rng.standard_normal((B, C_IN), dtype=np.float32),
        "W1": rng.standard_normal((C_IN, H), dtype=np.float32) * 0.02,
        "b1": np.full((H,), 0.1, np.float32),
        "Wg1": rng.standard_normal((H, G), dtype=np.float32) * 0.02,
        "bg1": np.full((G,), 0.1, np.float32),
        "Wg2": rng.standard_normal((G, E), dtype=np.float32) * 0.02,
        "bg2": np.full((E,), 0.1, np.float32),
        "We1": rng.standard_normal((E, H, D), dtype=np.float32) * 0.02,
        "be1": np.full((E, D), 0.1, np.float32),
        "We2": rng.standard_normal((E, D, D), dtype=np.float32) * 0.02,
        "be2": np.full((E, D), 0.1, np.float32),
    }
    out, ws, vs = kernel(**ins)
    print("out", out.shape, "ws", ws.shape, "vs", vs.shape)
